# revision 5
# baseline (speedup 1.0000x reference)
"""Trainium2 Bass kernel for nn_APCriterionWeighted (weighted-AP criterion).

Math summary (exact simplifications of the reference, not approximations):
  - sim_w = sim / stop_grad(sim * sim_self) == (1/sim_self) elementwise in
    real arithmetic (verified < 1.2e-7 rel diff in f32 on the fixed inputs).
  - x = 1/b for |b| <= 1 satisfies |x| >= 1, so in the 20-bin quantizer on
    [0, 1] every selected negative lands entirely in bin 0 (if b > 0) or
    bin 19 (if b < 0).  The per-row top-KNN of 1/b over a crop segment picks
    all positive-b entries first, so the negatives' soft-histogram is exactly
    [min(KNN, npos_seg) into bin 0, rest into bin 19] per segment.
  - Therefore per-row AP = f(diag terms, per-segment positive counts), where
    the counts come from the signs of sim_self = pos @ pos.T.

Device work per core (rows sharded 8 ways, data-parallel):
  - sim_self shard = pos_shard @ pos.T on PE (bf16 in, f32 PSUM out)
  - signs on ACT, sliding-window sign-sums + count epilogue + full 20-bin
    AP formula (cumsums, precision/recall, AP) on DVE
  - per-row AP DMA'd out; host computes the two scalar means (the unshard).
"""

import numpy as np
import ml_dtypes

KNN = 20


def _set_ap(ap, pairs):
    import bass_rust
    ap.ap = bass_rust.VecI64Pair(pairs)
    return ap
NQ = 20
N_CORES = 8
P = 128

# module knobs (test.py pokes these; the grading harness just calls kernel())
TRACE = False
LAST_EXEC_NS = None
LAST_TRACE_PATH = None
LAST_RESULTS = None

_GRAPH_CACHE = {}


def _crop_windows(kpts_crop_ids):
    """Replicate the reference's static segment walk.

    Returns list of (lo, n): off-diagonal columns [lo, lo+n) per active crop;
    in actual-column space the window is [lo, lo+n] (n+1 cols) with one
    excluded column clip(i, lo, lo+n) for row i.
    """
    kpts = np.asarray(kpts_crop_ids).astype(np.int64) - 1
    windows = []
    k = 0
    for n in kpts:
        n = int(n)
        if n < 0:
            continue
        if n < KNN:
            k += n
            continue
        windows.append((k, n))
        k += n
    return windows


def _quant_coeffs():
    a = np.float32(NQ - 1)
    w1 = np.full(NQ, -a, np.float32)
    b1 = np.arange(NQ, 0, -1).astype(np.float32)
    w2 = np.full(NQ, a, np.float32)
    b2 = np.arange(2 - NQ, 2, 1).astype(np.float32)
    w1[0] = 0.0
    b1[0] = 1.0
    w2[-1] = 0.0
    b2[-1] = 1.0
    return w1, b1, w2, b2


def _build_graph(b, d, windows):
    """Build the SPMD Bass/Tile graph (identical across cores)."""
    import concourse.bass as bass
    import concourse.tile as tile
    from concourse import bacc, mybir

    W = len(windows)
    S = b // N_CORES          # rows per core
    NT = S // P               # 128-row tiles per core
    NCH = (b + 511) // 512    # 512-col chunks of the full row
    assert S % P == 0 and b % 512 == 0

    uniform = len({n for _, n in windows}) == 1
    if uniform:
        n0 = windows[0][1]
        los = [lo for lo, _ in windows]
        steps = {los[i + 1] - los[i] for i in range(W - 1)} if W > 1 else {0}
        uniform = len(steps) <= 1
        lo_step = steps.pop() if W > 1 else 0

    f32 = mybir.dt.float32
    bf16 = mybir.dt.bfloat16

    nc = bacc.Bacc("TRN2", target_bir_lowering=False, debug=False,
                   enable_asserts=True, num_devices=N_CORES)

    posT_d = nc.declare_dram_parameter("posT", [P, b], bf16, isOutput=False)
    lhsT_d = nc.declare_dram_parameter("lhsT", [P, S], bf16, isOutput=False)
    anc_d = nc.declare_dram_parameter("anc_sh", [S, d], f32, isOutput=False)
    pos_d = nc.declare_dram_parameter("pos_sh", [S, d], f32, isOutput=False)
    bmask_d = nc.declare_dram_parameter("bmask", [P, 3 * NT * W], bf16, isOutput=False)
    fconst_d = nc.declare_dram_parameter("fconst", [P, NT * W + 4 * NQ], f32, isOutput=False)
    out_d = nc.declare_dram_parameter("out", [P, NT], f32, isOutput=True)

    with tile.TileContext(nc) as tc:
        with (
            tc.tile_pool(name="const", bufs=1) as cpool,
            tc.tile_pool(name="stage", bufs=4) as spool,
            tc.tile_pool(name="sgn", bufs=2) as gpool,
            tc.tile_pool(name="scr", bufs=2) as scrpool,
            tc.tile_pool(name="ep", bufs=1) as epool,
            tc.tile_pool(name="psum", bufs=8, space=bass.MemorySpace.PSUM) as ppool,
        ):
            # ---- constant loads ----
            posT = cpool.tile([P, b], bf16)
            for j in range(NCH):
                nc.sync.dma_start(posT[:, j * 512:(j + 1) * 512],
                                  posT_d.ap()[:, j * 512:(j + 1) * 512])
            lhsT = cpool.tile([P, S], bf16)
            nc.sync.dma_start(lhsT[:], lhsT_d.ap()[:])
            bmask = cpool.tile([P, 3 * NT * W], bf16)
            nc.sync.dma_start(bmask[:], bmask_d.ap()[:])
            fconst = cpool.tile([P, NT * W + 4 * NQ], f32)
            nc.sync.dma_start(fconst[:], fconst_d.ap()[:])

            # ---- diagonal path (f32, faithful a/(a*b)) ----
            pdiag = epool.tile([P, NT], f32)
            bdiag = epool.tile([P, NT], f32)
            for t in range(NT):
                a_t = spool.tile([P, d], f32, tag="diag_a")
                p_t = spool.tile([P, d], f32, tag="diag_p")
                nc.sync.dma_start(a_t[:], anc_d.ap()[t * P:(t + 1) * P, :])
                nc.sync.dma_start(p_t[:], pos_d.ap()[t * P:(t + 1) * P, :])
                scr = scrpool.tile([P, d], f32, tag="diag_scr")
                nc.vector.tensor_tensor(out=scr[:], in0=a_t[:], in1=p_t[:],
                                        op=mybir.AluOpType.mult)
                nc.vector.tensor_reduce(out=pdiag[:, t:t + 1], in_=scr[:],
                                        axis=mybir.AxisListType.X,
                                        op=mybir.AluOpType.add)
                scr2 = scrpool.tile([P, d], f32, tag="diag_scr")
                nc.vector.tensor_tensor(out=scr2[:], in0=p_t[:], in1=p_t[:],
                                        op=mybir.AluOpType.mult)
                nc.vector.tensor_reduce(out=bdiag[:, t:t + 1], in_=scr2[:],
                                        axis=mybir.AxisListType.X,
                                        op=mybir.AluOpType.add)
            pbprod = epool.tile([P, NT], f32)
            nc.vector.tensor_tensor(out=pbprod[:], in0=pdiag[:], in1=bdiag[:],
                                    op=mybir.AluOpType.mult)
            pbinv = epool.tile([P, NT], f32)
            nc.vector.reciprocal(pbinv[:], pbprod[:])
            pval = epool.tile([P, NT], f32)
            nc.vector.tensor_tensor(out=pval[:], in0=pdiag[:], in1=pbinv[:],
                                    op=mybir.AluOpType.mult)

            # ---- main loop: matmul -> sign -> window sign-sums -> corr ----
            ssum = epool.tile([P, NT * W], f32)
            corr = epool.tile([P, NT * W], f32)
            c1 = scrpool.tile([P, NT * W], f32)
            c2 = scrpool.tile([P, NT * W], f32)
            for t in range(NT):
                sgn_t = gpool.tile([P, b], bf16, tag="sgn")
                for j in range(NCH):
                    ps = ppool.tile([P, 512], f32)
                    nc.tensor.matmul(ps[:], lhsT[:, t * P:(t + 1) * P],
                                     posT[:, j * 512:(j + 1) * 512],
                                     start=True, stop=True)
                    nc.scalar.sign(sgn_t[:, j * 512:(j + 1) * 512], ps[:])

                # sliding-window sign sums
                if uniform:
                    win = sgn_t[:].copy()
                    win.offset = win.offset + los[0]
                    _set_ap(win, [tuple(win.ap[0]), (max(lo_step, 1), W), (1, n0 + 1)])
                    nc.vector.tensor_reduce(
                        out=ssum[:, t * W:(t + 1) * W], in_=win,
                        axis=mybir.AxisListType.X, op=mybir.AluOpType.add)
                else:
                    for w, (lo, n) in enumerate(windows):
                        nc.vector.tensor_reduce(
                            out=ssum[:, t * W + w:t * W + w + 1],
                            in_=sgn_t[:, lo:lo + n + 1],
                            axis=mybir.AxisListType.X, op=mybir.AluOpType.add)

                # corr = A*sgn[lo] + B*sgn[hi] + D   per window
                if uniform:
                    lo_v = sgn_t[:].copy()
                    lo_v.offset = lo_v.offset + los[0]
                    _set_ap(lo_v, [tuple(lo_v.ap[0]), (max(lo_step, 1), W)])
                    hi_v = sgn_t[:].copy()
                    hi_v.offset = hi_v.offset + los[0] + n0
                    _set_ap(hi_v, [tuple(hi_v.ap[0]), (max(lo_step, 1), W)])
                else:
                    # gather columns one by one into a packed scratch
                    lo_pack = scrpool.tile([P, W], bf16, tag="lopack")
                    hi_pack = scrpool.tile([P, W], bf16, tag="hipack")
                    for w, (lo, n) in enumerate(windows):
                        nc.vector.tensor_copy(lo_pack[:, w:w + 1], sgn_t[:, lo:lo + 1])
                        nc.vector.tensor_copy(hi_pack[:, w:w + 1], sgn_t[:, lo + n:lo + n + 1])
                    lo_v = lo_pack[:]
                    hi_v = hi_pack[:]
                tw = slice(t * W, (t + 1) * W)
                nc.vector.tensor_tensor(out=c1[:, tw], in0=bmask[:, t * W:(t + 1) * W],
                                        in1=lo_v, op=mybir.AluOpType.mult)
                nc.vector.tensor_tensor(out=c2[:, tw],
                                        in0=bmask[:, NT * W + t * W:NT * W + (t + 1) * W],
                                        in1=hi_v, op=mybir.AluOpType.mult)
                nc.vector.tensor_tensor(out=c1[:, tw], in0=c1[:, tw], in1=c2[:, tw],
                                        op=mybir.AluOpType.add)
                # + D (bf16 mask -> f32 add via copy-widen through c2)
                nc.vector.tensor_copy(c2[:, tw], bmask[:, 2 * NT * W + t * W:2 * NT * W + (t + 1) * W])
                nc.vector.tensor_tensor(out=corr[:, tw], in0=c1[:, tw], in1=c2[:, tw],
                                        op=mybir.AluOpType.add)

            # ---- counts ----
            raw = epool.tile([P, NT * W], f32)
            nc.vector.tensor_tensor(out=raw[:], in0=ssum[:], in1=corr[:],
                                    op=mybir.AluOpType.subtract)
            npos = epool.tile([P, NT * W], f32)
            halfn_v = fconst[:, 0:NT * W]
            nc.vector.scalar_tensor_tensor(out=npos[:], in0=raw[:], scalar=0.5,
                                           in1=halfn_v, op0=mybir.AluOpType.mult,
                                           op1=mybir.AluOpType.add)
            nc.vector.tensor_scalar_min(npos[:], npos[:], float(KNN))
            m0 = epool.tile([P, NT], f32)
            nc.vector.tensor_reduce(out=m0[:], in_=npos[:].rearrange("p (t w) -> p t w", w=W),
                                    axis=mybir.AxisListType.X, op=mybir.AluOpType.add)
            m19 = epool.tile([P, NT], f32)
            nc.vector.tensor_scalar(out=m19[:], in0=m0[:], scalar1=-1.0,
                                    scalar2=float(KNN * W), op0=mybir.AluOpType.mult,
                                    op1=mybir.AluOpType.add)

            # ---- rec = psi_j(p)  [P, NT*NQ] ----
            qoff = NT * W
            def quant_bc(k):
                v = fconst[:].copy()
                v.offset = v.offset + qoff + k * NQ
                _set_ap(v, [tuple(v.ap[0]), (0, NT), (1, NQ)])
                return v
            pbc = pval[:].copy()
            _set_ap(pbc, [tuple(pbc.ap[0]), (1, NT), (0, NQ)])

            q1 = epool.tile([P, NT * NQ], f32)
            q2 = epool.tile([P, NT * NQ], f32)
            rec = epool.tile([P, NT * NQ], f32)
            nbs = epool.tile([P, NT * NQ], f32)
            q1v = q1[:].rearrange("p (t q) -> p t q", q=NQ)
            q2v = q2[:].rearrange("p (t q) -> p t q", q=NQ)
            nc.vector.tensor_tensor(out=q1v, in0=pbc, in1=quant_bc(0), op=mybir.AluOpType.mult)
            nc.vector.tensor_tensor(out=q1v, in0=q1v, in1=quant_bc(1), op=mybir.AluOpType.add)
            nc.vector.tensor_tensor(out=q2v, in0=pbc, in1=quant_bc(2), op=mybir.AluOpType.mult)
            nc.vector.tensor_tensor(out=q2v, in0=q2v, in1=quant_bc(3), op=mybir.AluOpType.add)
            nc.vector.tensor_tensor(out=q1[:], in0=q1[:], in1=q2[:], op=mybir.AluOpType.min)
            nc.vector.tensor_scalar_max(rec[:], q1[:], 0.0)

            nc.vector.tensor_copy(nbs[:], rec[:])
            nbs0 = nbs[:, 0:NT * NQ:NQ]
            nc.vector.tensor_tensor(out=nbs0, in0=nbs0, in1=m0[:], op=mybir.AluOpType.add)
            nbs19 = nbs[:, NQ - 1:NT * NQ:NQ]
            nc.vector.tensor_tensor(out=nbs19, in0=nbs19, in1=m19[:], op=mybir.AluOpType.add)

            # ---- cumsums, prec, ap ----
            cumr = epool.tile([P, NT * NQ], f32)
            cumn = epool.tile([P, NT * NQ], f32)
            for t in range(NT):
                sl = slice(t * NQ, (t + 1) * NQ)
                nc.vector.tensor_tensor_scan(
                    out=cumr[:, sl], data0=rec[:, sl], data1=rec[:, sl],
                    initial=0.0, op0=mybir.AluOpType.add, op1=mybir.AluOpType.bypass)
                nc.vector.tensor_tensor_scan(
                    out=cumn[:, sl], data0=nbs[:, sl], data1=nbs[:, sl],
                    initial=0.0, op0=mybir.AluOpType.add, op1=mybir.AluOpType.bypass)
            nc.vector.tensor_scalar_add(cumn[:], cumn[:], 1e-16)
            cninv = epool.tile([P, NT * NQ], f32)
            nc.vector.reciprocal(cninv[:], cumn[:])
            prec = epool.tile([P, NT * NQ], f32)
            nc.vector.tensor_tensor(out=prec[:], in0=cumr[:], in1=cninv[:],
                                    op=mybir.AluOpType.mult)

            srec = epool.tile([P, NT], f32)
            nc.vector.tensor_reduce(out=srec[:], in_=rec[:].rearrange("p (t q) -> p t q", q=NQ),
                                    axis=mybir.AxisListType.X, op=mybir.AluOpType.add)
            sinv = epool.tile([P, NT], f32)
            nc.vector.reciprocal(sinv[:], srec[:])

            apraw = epool.tile([P, NT], f32)
            apterm = epool.tile([P, NT * NQ], f32)
            nc.vector.tensor_tensor(out=apterm[:], in0=prec[:], in1=rec[:],
                                    op=mybir.AluOpType.mult)
            nc.vector.tensor_reduce(out=apraw[:],
                                    in_=apterm[:].rearrange("p (t q) -> p t q", q=NQ),
                                    axis=mybir.AxisListType.X, op=mybir.AluOpType.add)
            apout = epool.tile([P, NT], f32)
            nc.vector.tensor_tensor(out=apout[:], in0=apraw[:], in1=sinv[:],
                                    op=mybir.AluOpType.mult)
            nc.sync.dma_start(out_d.ap()[:], apout[:])

    nc.compile()
    return nc


def _host_inputs(anc, pos, windows, b, d):
    """Per-core input maps (the sharding step)."""
    W = len(windows)
    S = b // N_CORES
    NT = S // P
    w1, b1, w2, b2 = _quant_coeffs()

    pos_bf = pos.astype(ml_dtypes.bfloat16)
    posT = np.ascontiguousarray(pos_bf.T)                     # [d, b] bf16

    quant = np.concatenate([w1, b1, w2, b2]).astype(np.float32)  # [4*NQ]

    in_maps = []
    for c in range(N_CORES):
        rows = np.arange(c * S, (c + 1) * S)
        # masks per (row, window)
        A = np.zeros((S, W), np.float32)
        B = np.zeros((S, W), np.float32)
        D = np.zeros((S, W), np.float32)
        halfn = np.zeros((S, W), np.float32)
        for w, (lo, n) in enumerate(windows):
            hi = lo + n
            A[:, w] = rows < lo
            B[:, w] = rows > hi
            D[:, w] = (rows >= lo) & (rows <= hi)
            halfn[:, w] = n / 2.0

        def to_ptw(x):  # [S, W] -> [P, NT*W]
            return np.ascontiguousarray(
                x.reshape(NT, P, W).transpose(1, 0, 2).reshape(P, NT * W))

        bmask = np.concatenate([to_ptw(A), to_ptw(B), to_ptw(D)], axis=1)
        fconst = np.concatenate(
            [to_ptw(halfn), np.tile(quant[None, :], (P, 1))], axis=1).astype(np.float32)

        in_maps.append({
            "posT": posT,
            "lhsT": np.ascontiguousarray(pos_bf[c * S:(c + 1) * S].T),
            "anc_sh": np.ascontiguousarray(anc[c * S:(c + 1) * S]),
            "pos_sh": np.ascontiguousarray(pos[c * S:(c + 1) * S]),
            "bmask": bmask.astype(ml_dtypes.bfloat16),
            "fconst": fconst,
        })
    return in_maps


def kernel(anc_feat, pos_feat, kpts_crop_ids):
    global LAST_EXEC_NS, LAST_TRACE_PATH, LAST_RESULTS
    from concourse.bass_utils import run_bass_kernel_spmd

    anc = np.asarray(anc_feat, dtype=np.float32)
    pos = np.asarray(pos_feat, dtype=np.float32)
    b, d = pos.shape
    windows = _crop_windows(kpts_crop_ids)
    W = len(windows)
    S = b // N_CORES
    NT = S // P

    key = (b, d, tuple(windows))
    if key not in _GRAPH_CACHE:
        _GRAPH_CACHE[key] = _build_graph(b, d, windows)
    nc = _GRAPH_CACHE[key]

    in_maps = _host_inputs(anc, pos, windows, b, d)
    res = run_bass_kernel_spmd(nc, in_maps, list(range(N_CORES)), trace=TRACE)
    LAST_RESULTS = res
    LAST_EXEC_NS = res.exec_time_ns
    if res.instructions_and_trace is not None:
        LAST_TRACE_PATH = res.instructions_and_trace[1]

    ap = np.empty(b, np.float32)
    for c in range(N_CORES):
        o = np.asarray(res.results[c]["out"], dtype=np.float32)  # [P, NT]
        ap[c * S:(c + 1) * S] = o.T.reshape(S)

    one = np.float32(1.0)
    loss = (one - ap).mean(dtype=np.float32)
    apm = ap.mean(dtype=np.float32)
    return (np.asarray(loss, dtype=np.float32), np.asarray(apm, dtype=np.float32))


# revision 8
# speedup vs baseline: 1.0676x; 1.0676x over previous
"""Trainium2 Bass kernel for nn_APCriterionWeighted (weighted-AP criterion).

Math summary (exact simplifications of the reference, not approximations):
  - sim_w = sim / stop_grad(sim * sim_self) == (1/sim_self) elementwise in
    real arithmetic (verified < 1.2e-7 rel diff in f32 on the fixed inputs).
  - x = 1/b for |b| <= 1 satisfies |x| >= 1, so in the 20-bin quantizer on
    [0, 1] every selected negative lands entirely in bin 0 (if b > 0) or
    bin 19 (if b < 0).  The per-row top-KNN of 1/b over a crop segment picks
    all positive-b entries first, so the negatives' soft-histogram is exactly
    [min(KNN, npos_seg) into bin 0, rest into bin 19] per segment.
  - Therefore per-row AP = f(diag terms, per-segment positive counts), where
    the counts come from the signs of sim_self = pos @ pos.T.

Device work per core (rows sharded 8 ways, data-parallel):
  - sim_self shard = pos_shard @ pos.T on PE (bf16 in, f32 PSUM out)
  - signs on ACT, sliding-window sign-sums + count epilogue + full 20-bin
    AP formula (cumsums, precision/recall, AP) on DVE
  - per-row AP DMA'd out; host computes the two scalar means (the unshard).
"""

import numpy as np
import ml_dtypes

KNN = 20


def _set_ap(ap, pairs):
    import bass_rust
    ap.ap = bass_rust.VecI64Pair(pairs)
    return ap
NQ = 20
N_CORES = 8
P = 128

# module knobs (test.py pokes these; the grading harness just calls kernel())
TRACE = False
LAST_EXEC_NS = None
LAST_TRACE_PATH = None
LAST_RESULTS = None

_GRAPH_CACHE = {}


def _crop_windows(kpts_crop_ids):
    """Replicate the reference's static segment walk.

    Returns list of (lo, n): off-diagonal columns [lo, lo+n) per active crop;
    in actual-column space the window is [lo, lo+n] (n+1 cols) with one
    excluded column clip(i, lo, lo+n) for row i.
    """
    kpts = np.asarray(kpts_crop_ids).astype(np.int64) - 1
    windows = []
    k = 0
    for n in kpts:
        n = int(n)
        if n < 0:
            continue
        if n < KNN:
            k += n
            continue
        windows.append((k, n))
        k += n
    return windows


def _quant_coeffs():
    a = np.float32(NQ - 1)
    w1 = np.full(NQ, -a, np.float32)
    b1 = np.arange(NQ, 0, -1).astype(np.float32)
    w2 = np.full(NQ, a, np.float32)
    b2 = np.arange(2 - NQ, 2, 1).astype(np.float32)
    w1[0] = 0.0
    b1[0] = 1.0
    w2[-1] = 0.0
    b2[-1] = 1.0
    return w1, b1, w2, b2


def _build_graph(b, d, windows):
    """Build the SPMD Bass/Tile graph (identical across cores)."""
    import concourse.bass as bass
    import concourse.tile as tile
    from concourse import bacc, mybir

    W = len(windows)
    S = b // N_CORES          # rows per core
    NT = S // P               # 128-row tiles per core
    NCH = (b + 511) // 512    # 512-col chunks of the full row
    assert S % P == 0 and b % 512 == 0

    uniform = len({n for _, n in windows}) == 1
    if uniform:
        n0 = windows[0][1]
        los = [lo for lo, _ in windows]
        steps = {los[i + 1] - los[i] for i in range(W - 1)} if W > 1 else {0}
        uniform = len(steps) <= 1
        lo_step = steps.pop() if W > 1 else 0

    f32 = mybir.dt.float32
    bf16 = mybir.dt.bfloat16

    nc = bacc.Bacc("TRN2", target_bir_lowering=False, debug=False,
                   enable_asserts=True, num_devices=N_CORES)

    posT_d = nc.declare_dram_parameter("posT", [P, b], bf16, isOutput=False)
    lhsT_d = nc.declare_dram_parameter("lhsT", [P, S], bf16, isOutput=False)
    anc_d = nc.declare_dram_parameter("anc_sh", [S, d], f32, isOutput=False)
    pos_d = nc.declare_dram_parameter("pos_sh", [S, d], f32, isOutput=False)
    bmask_d = nc.declare_dram_parameter("bmask", [P, 3 * NT * W], bf16, isOutput=False)
    fconst_d = nc.declare_dram_parameter("fconst", [P, NT * W + 4 * NQ], f32, isOutput=False)
    out_d = nc.declare_dram_parameter("out", [P, NT], f32, isOutput=True)

    with tile.TileContext(nc) as tc:
        with (
            tc.tile_pool(name="const", bufs=1) as cpool,
            tc.tile_pool(name="stage", bufs=4) as spool,
            tc.tile_pool(name="sgn", bufs=2) as gpool,
            tc.tile_pool(name="scr", bufs=2) as scrpool,
            tc.tile_pool(name="ep", bufs=1) as epool,
            tc.tile_pool(name="psum", bufs=8, space=bass.MemorySpace.PSUM) as ppool,
        ):
            # ---- constant loads ----
            posT = cpool.tile([P, b], bf16)
            for j in range(NCH):
                nc.sync.dma_start(posT[:, j * 512:(j + 1) * 512],
                                  posT_d.ap()[:, j * 512:(j + 1) * 512])
            lhsT = cpool.tile([P, S], bf16)
            nc.sync.dma_start(lhsT[:], lhsT_d.ap()[:])
            bmask = cpool.tile([P, 3 * NT * W], bf16)
            nc.sync.dma_start(bmask[:], bmask_d.ap()[:])
            fconst = cpool.tile([P, NT * W + 4 * NQ], f32)
            nc.sync.dma_start(fconst[:], fconst_d.ap()[:])

            # ---- diagonal path (f32, faithful a/(a*b)) ----
            pdiag = epool.tile([P, NT], f32)
            bdiag = epool.tile([P, NT], f32)
            for t in range(NT):
                a_t = spool.tile([P, d], f32, tag="diag_a")
                p_t = spool.tile([P, d], f32, tag="diag_p")
                nc.sync.dma_start(a_t[:], anc_d.ap()[t * P:(t + 1) * P, :])
                nc.sync.dma_start(p_t[:], pos_d.ap()[t * P:(t + 1) * P, :])
                scr = scrpool.tile([P, d], f32, tag="diag_scr")
                nc.vector.tensor_tensor(out=scr[:], in0=a_t[:], in1=p_t[:],
                                        op=mybir.AluOpType.mult)
                nc.vector.tensor_reduce(out=pdiag[:, t:t + 1], in_=scr[:],
                                        axis=mybir.AxisListType.X,
                                        op=mybir.AluOpType.add)
                scr2 = scrpool.tile([P, d], f32, tag="diag_scr")
                nc.vector.tensor_tensor(out=scr2[:], in0=p_t[:], in1=p_t[:],
                                        op=mybir.AluOpType.mult)
                nc.vector.tensor_reduce(out=bdiag[:, t:t + 1], in_=scr2[:],
                                        axis=mybir.AxisListType.X,
                                        op=mybir.AluOpType.add)
            pbprod = epool.tile([P, NT], f32)
            nc.vector.tensor_tensor(out=pbprod[:], in0=pdiag[:], in1=bdiag[:],
                                    op=mybir.AluOpType.mult)
            pbinv = epool.tile([P, NT], f32)
            nc.vector.reciprocal(pbinv[:], pbprod[:])
            pval = epool.tile([P, NT], f32)
            nc.vector.tensor_tensor(out=pval[:], in0=pdiag[:], in1=pbinv[:],
                                    op=mybir.AluOpType.mult)

            # ---- main loop: matmul -> sign -> window sign-sums -> corr ----
            ssum = epool.tile([P, NT * W], f32)
            corr = epool.tile([P, NT * W], f32)
            c1 = scrpool.tile([P, NT * W], f32)
            c2 = scrpool.tile([P, NT * W], f32)
            for t in range(NT):
                sgn_t = gpool.tile([P, b], bf16, tag="sgn")
                for j in range(NCH):
                    ps = ppool.tile([P, 512], f32)
                    nc.tensor.matmul(ps[:], lhsT[:, t * P:(t + 1) * P],
                                     posT[:, j * 512:(j + 1) * 512],
                                     start=True, stop=True)
                    nc.scalar.sign(sgn_t[:, j * 512:(j + 1) * 512], ps[:])

                # sliding-window sign sums
                if uniform:
                    win = sgn_t[:].copy()
                    win.offset = win.offset + los[0]
                    _set_ap(win, [tuple(win.ap[0]), (max(lo_step, 1), W), (1, n0 + 1)])
                    nc.vector.tensor_reduce(
                        out=ssum[:, t * W:(t + 1) * W], in_=win,
                        axis=mybir.AxisListType.X, op=mybir.AluOpType.add)
                else:
                    for w, (lo, n) in enumerate(windows):
                        nc.vector.tensor_reduce(
                            out=ssum[:, t * W + w:t * W + w + 1],
                            in_=sgn_t[:, lo:lo + n + 1],
                            axis=mybir.AxisListType.X, op=mybir.AluOpType.add)

                # corr = A*sgn[lo] + B*sgn[hi] + D   per window
                if uniform:
                    lo_v = sgn_t[:].copy()
                    lo_v.offset = lo_v.offset + los[0]
                    _set_ap(lo_v, [tuple(lo_v.ap[0]), (max(lo_step, 1), W)])
                    hi_v = sgn_t[:].copy()
                    hi_v.offset = hi_v.offset + los[0] + n0
                    _set_ap(hi_v, [tuple(hi_v.ap[0]), (max(lo_step, 1), W)])
                else:
                    # gather columns one by one into a packed scratch
                    lo_pack = scrpool.tile([P, W], bf16, tag="lopack")
                    hi_pack = scrpool.tile([P, W], bf16, tag="hipack")
                    for w, (lo, n) in enumerate(windows):
                        nc.vector.tensor_copy(lo_pack[:, w:w + 1], sgn_t[:, lo:lo + 1])
                        nc.vector.tensor_copy(hi_pack[:, w:w + 1], sgn_t[:, lo + n:lo + n + 1])
                    lo_v = lo_pack[:]
                    hi_v = hi_pack[:]
                tw = slice(t * W, (t + 1) * W)
                nc.vector.tensor_tensor(out=c1[:, tw], in0=bmask[:, t * W:(t + 1) * W],
                                        in1=lo_v, op=mybir.AluOpType.mult)
                nc.vector.tensor_tensor(out=c2[:, tw],
                                        in0=bmask[:, NT * W + t * W:NT * W + (t + 1) * W],
                                        in1=hi_v, op=mybir.AluOpType.mult)
                nc.vector.tensor_tensor(out=c1[:, tw], in0=c1[:, tw], in1=c2[:, tw],
                                        op=mybir.AluOpType.add)
                # + D (bf16 mask -> f32 add via copy-widen through c2)
                nc.vector.tensor_copy(c2[:, tw], bmask[:, 2 * NT * W + t * W:2 * NT * W + (t + 1) * W])
                nc.vector.tensor_tensor(out=corr[:, tw], in0=c1[:, tw], in1=c2[:, tw],
                                        op=mybir.AluOpType.add)

            # ---- counts ----
            raw = epool.tile([P, NT * W], f32)
            nc.vector.tensor_tensor(out=raw[:], in0=ssum[:], in1=corr[:],
                                    op=mybir.AluOpType.subtract)
            npos = epool.tile([P, NT * W], f32)
            halfn_v = fconst[:, 0:NT * W]
            nc.vector.scalar_tensor_tensor(out=npos[:], in0=raw[:], scalar=0.5,
                                           in1=halfn_v, op0=mybir.AluOpType.mult,
                                           op1=mybir.AluOpType.add)
            nc.vector.tensor_scalar_min(npos[:], npos[:], float(KNN))
            m0 = epool.tile([P, NT], f32)
            nc.vector.tensor_reduce(out=m0[:], in_=npos[:].rearrange("p (t w) -> p t w", w=W),
                                    axis=mybir.AxisListType.X, op=mybir.AluOpType.add)
            m19 = epool.tile([P, NT], f32)
            nc.vector.tensor_scalar(out=m19[:], in0=m0[:], scalar1=-1.0,
                                    scalar2=float(KNN * W), op0=mybir.AluOpType.mult,
                                    op1=mybir.AluOpType.add)

            # ---- rec = psi_j(p)  [P, NT*NQ] ----
            qoff = NT * W
            def quant_bc(k):
                v = fconst[:].copy()
                v.offset = v.offset + qoff + k * NQ
                _set_ap(v, [tuple(v.ap[0]), (0, NT), (1, NQ)])
                return v
            pbc = pval[:].copy()
            _set_ap(pbc, [tuple(pbc.ap[0]), (1, NT), (0, NQ)])

            q1 = epool.tile([P, NT * NQ], f32)
            q2 = epool.tile([P, NT * NQ], f32)
            rec = epool.tile([P, NT * NQ], f32)
            nbs = epool.tile([P, NT * NQ], f32)
            q1v = q1[:].rearrange("p (t q) -> p t q", q=NQ)
            q2v = q2[:].rearrange("p (t q) -> p t q", q=NQ)
            nc.vector.tensor_tensor(out=q1v, in0=pbc, in1=quant_bc(0), op=mybir.AluOpType.mult)
            nc.vector.tensor_tensor(out=q1v, in0=q1v, in1=quant_bc(1), op=mybir.AluOpType.add)
            nc.vector.tensor_tensor(out=q2v, in0=pbc, in1=quant_bc(2), op=mybir.AluOpType.mult)
            nc.vector.tensor_tensor(out=q2v, in0=q2v, in1=quant_bc(3), op=mybir.AluOpType.add)
            nc.vector.tensor_tensor(out=q1[:], in0=q1[:], in1=q2[:], op=mybir.AluOpType.min)
            nc.vector.tensor_scalar_max(rec[:], q1[:], 0.0)

            nc.vector.tensor_copy(nbs[:], rec[:])
            nbs0 = nbs[:, 0:NT * NQ:NQ]
            nc.vector.tensor_tensor(out=nbs0, in0=nbs0, in1=m0[:], op=mybir.AluOpType.add)
            nbs19 = nbs[:, NQ - 1:NT * NQ:NQ]
            nc.vector.tensor_tensor(out=nbs19, in0=nbs19, in1=m19[:], op=mybir.AluOpType.add)

            # ---- cumsums, prec, ap ----
            cumr = epool.tile([P, NT * NQ], f32)
            cumn = epool.tile([P, NT * NQ], f32)
            for t in range(NT):
                sl = slice(t * NQ, (t + 1) * NQ)
                nc.vector.tensor_tensor_scan(
                    out=cumr[:, sl], data0=rec[:, sl], data1=rec[:, sl],
                    initial=0.0, op0=mybir.AluOpType.add, op1=mybir.AluOpType.bypass)
                nc.vector.tensor_tensor_scan(
                    out=cumn[:, sl], data0=nbs[:, sl], data1=nbs[:, sl],
                    initial=0.0, op0=mybir.AluOpType.add, op1=mybir.AluOpType.bypass)
            nc.vector.tensor_scalar_add(cumn[:], cumn[:], 1e-16)
            cninv = epool.tile([P, NT * NQ], f32)
            nc.vector.reciprocal(cninv[:], cumn[:])
            prec = epool.tile([P, NT * NQ], f32)
            nc.vector.tensor_tensor(out=prec[:], in0=cumr[:], in1=cninv[:],
                                    op=mybir.AluOpType.mult)

            srec = epool.tile([P, NT], f32)
            nc.vector.tensor_reduce(out=srec[:], in_=rec[:].rearrange("p (t q) -> p t q", q=NQ),
                                    axis=mybir.AxisListType.X, op=mybir.AluOpType.add)
            sinv = epool.tile([P, NT], f32)
            nc.vector.reciprocal(sinv[:], srec[:])

            apraw = epool.tile([P, NT], f32)
            apterm = epool.tile([P, NT * NQ], f32)
            nc.vector.tensor_tensor(out=apterm[:], in0=prec[:], in1=rec[:],
                                    op=mybir.AluOpType.mult)
            nc.vector.tensor_reduce(out=apraw[:],
                                    in_=apterm[:].rearrange("p (t q) -> p t q", q=NQ),
                                    axis=mybir.AxisListType.X, op=mybir.AluOpType.add)
            apout = epool.tile([P, NT], f32)
            nc.vector.tensor_tensor(out=apout[:], in0=apraw[:], in1=sinv[:],
                                    op=mybir.AluOpType.mult)
            nc.sync.dma_start(out_d.ap()[:], apout[:])

    nc.compile()
    return nc



def _build_graph_v2(b, d, windows, act_tiles):
    """Transposed-counts design (uniform windows, width n+1 = 256, lo step 255).

    Per core: Gt col-tiles [128 cols, S rows] on PE; sign(ACT)/is_gt(DVE) per
    col-tile; per-window positive counts via a selector matmul on PE
    (contraction over the col partitions), accumulated in one PSUM bank;
    boundary-column corrections from a tiny strided matmul; epilogue row-major.
    """
    import concourse.bass as bass
    import concourse.tile as tile
    from concourse import bacc, mybir

    W = len(windows)
    S = b // N_CORES
    NT = S // P
    NCT = b // P                  # col-tiles
    n0 = windows[0][1]
    lo0 = windows[0][0]
    lo_step = windows[1][0] - windows[0][0] if W > 1 else 1
    NB = W + 1                    # boundary cols (shared lo/hi)

    f32 = mybir.dt.float32
    bf16 = mybir.dt.bfloat16

    nc = bacc.Bacc("TRN2", target_bir_lowering=False, debug=False,
                   enable_asserts=True, num_devices=N_CORES)

    posT_d = nc.declare_dram_parameter("posT", [P, b], bf16, isOutput=False)
    lhsT_d = nc.declare_dram_parameter("lhsT", [P, S], bf16, isOutput=False)
    selw_d = nc.declare_dram_parameter("selw", [P, NCT * W], bf16, isOutput=False)
    anc_d = nc.declare_dram_parameter("anc_sh", [S, d], f32, isOutput=False)
    pos_d = nc.declare_dram_parameter("pos_sh", [S, d], f32, isOutput=False)
    bmask_d = nc.declare_dram_parameter("bmask", [P, 3 * NT * W], bf16, isOutput=False)
    fconst_d = nc.declare_dram_parameter("fconst", [P, W + 4 * NQ], f32, isOutput=False)
    fid_d = nc.declare_dram_parameter("fid16", [W, W], f32, isOutput=False)
    out_d = nc.declare_dram_parameter("out", [P, NT], f32, isOutput=True)

    with tile.TileContext(nc) as tc:
        with (
            tc.tile_pool(name="const", bufs=1) as cpool,
            tc.tile_pool(name="stage", bufs=4) as spool,
            tc.tile_pool(name="sgn", bufs=6) as gpool,
            tc.tile_pool(name="scr", bufs=2) as scrpool,
            tc.tile_pool(name="ep", bufs=1) as epool,
            tc.tile_pool(name="psum", bufs=5, space=bass.MemorySpace.PSUM) as ppool,
            tc.tile_pool(name="psacc", bufs=1, space=bass.MemorySpace.PSUM) as papool,
            tc.tile_pool(name="pssm", bufs=1, space=bass.MemorySpace.PSUM) as pspool,
        ):
            # ---- input loads (posT/lhsT/selw first: they gate the PE) ----
            posT = cpool.tile([P, b], bf16)
            for j in range(8):
                cw = b // 8
                nc.sync.dma_start(posT[:, j * cw:(j + 1) * cw],
                                  posT_d.ap()[:, j * cw:(j + 1) * cw])
            lhsT = cpool.tile([P, S], bf16)
            for j in range(2):
                nc.sync.dma_start(lhsT[:, j * (S // 2):(j + 1) * (S // 2)],
                                  lhsT_d.ap()[:, j * (S // 2):(j + 1) * (S // 2)])
            selw = cpool.tile([P, NCT * W], bf16)
            for j in range(4):
                cw = NCT * W // 4
                nc.scalar.dma_start(selw[:, j * cw:(j + 1) * cw],
                                    selw_d.ap()[:, j * cw:(j + 1) * cw])
            bmask = cpool.tile([P, 3 * NT * W], bf16)
            nc.scalar.dma_start(bmask[:], bmask_d.ap()[:])
            fconst = cpool.tile([P, W + 4 * NQ], f32)
            nc.scalar.dma_start(fconst[:], fconst_d.ap()[:])
            fid = cpool.tile([W, W], f32)
            nc.scalar.dma_start(fid[:], fid_d.ap()[:])

            # ---- main col-tile loop: Gt -> sign/ind -> selector matmul ----
            ssumT_ps = papool.tile([W, S], f32)
            for ct in range(NCT):
                ps = ppool.tile([P, S], f32)
                nc.tensor.matmul(ps[:], posT[:, ct * P:(ct + 1) * P], lhsT[:],
                                 start=True, stop=True)
                v_ct = gpool.tile([P, S], bf16, tag="sgnT")
                if ct in act_tiles:
                    nc.scalar.sign(v_ct[:], ps[:])
                else:
                    nc.vector.tensor_scalar(out=v_ct[:], in0=ps[:], scalar1=0.0,
                                            scalar2=None, op0=mybir.AluOpType.is_gt)
                nc.tensor.matmul(ssumT_ps[:], selw[:, ct * W:(ct + 1) * W], v_ct[:],
                                 start=(ct == 0), stop=(ct == NCT - 1))

            # ---- boundary columns (row-major, tiny strided matmul) ----
            bndv = posT[:].copy()
            bndv.offset = bndv.offset + lo0
            _set_ap(bndv, [tuple(bndv.ap[0]), (lo_step, NB)])
            bnd_ind = epool.tile([P, NT * NB], bf16)
            for t in range(NT):
                bps = pspool.tile([P, NB], f32, tag="bnd")
                nc.tensor.matmul(bps[:], lhsT[:, t * P:(t + 1) * P], bndv,
                                 start=True, stop=True)
                nc.vector.tensor_scalar(out=bnd_ind[:, t * NB:(t + 1) * NB],
                                        in0=bps[:], scalar1=0.0, scalar2=None,
                                        op0=mybir.AluOpType.is_gt)

            # ---- counts back to row-major: PSUM -> SBUF -> PE transpose ----
            ssumT_sb = epool.tile([W, S], f32)
            nc.vector.tensor_copy(ssumT_sb[:], ssumT_ps[:])
            ssum = epool.tile([P, NT * W], f32)
            for t in range(NT):
                tps = pspool.tile([P, W], f32, tag="ctr")
                nc.tensor.transpose(tps[:], ssumT_sb[:, t * P:(t + 1) * P], fid[:])
                nc.vector.tensor_copy(ssum[:, t * W:(t + 1) * W], tps[:])

            # ---- diagonal path (f32, faithful a/(a*b)) ----
            pdiag = epool.tile([P, NT], f32)
            bdiag = epool.tile([P, NT], f32)
            for t in range(NT):
                a_t = spool.tile([P, d], f32, tag="diag_a")
                p_t = spool.tile([P, d], f32, tag="diag_p")
                nc.sync.dma_start(a_t[:], anc_d.ap()[t * P:(t + 1) * P, :])
                nc.sync.dma_start(p_t[:], pos_d.ap()[t * P:(t + 1) * P, :])
                scr = scrpool.tile([P, d], f32, tag="diag_scr")
                nc.vector.tensor_tensor(out=scr[:], in0=a_t[:], in1=p_t[:],
                                        op=mybir.AluOpType.mult)
                nc.vector.tensor_reduce(out=pdiag[:, t:t + 1], in_=scr[:],
                                        axis=mybir.AxisListType.X,
                                        op=mybir.AluOpType.add)
                scr2 = scrpool.tile([P, d], f32, tag="diag_scr")
                nc.vector.tensor_tensor(out=scr2[:], in0=p_t[:], in1=p_t[:],
                                        op=mybir.AluOpType.mult)
                nc.vector.tensor_reduce(out=bdiag[:, t:t + 1], in_=scr2[:],
                                        axis=mybir.AxisListType.X,
                                        op=mybir.AluOpType.add)
            pbprod = epool.tile([P, NT], f32)
            nc.vector.tensor_tensor(out=pbprod[:], in0=pdiag[:], in1=bdiag[:],
                                    op=mybir.AluOpType.mult)
            pbinv = epool.tile([P, NT], f32)
            nc.vector.reciprocal(pbinv[:], pbprod[:])
            pval = epool.tile([P, NT], f32)
            nc.vector.tensor_tensor(out=pval[:], in0=pdiag[:], in1=pbinv[:],
                                    op=mybir.AluOpType.mult)

            # ---- corr = A*ind[lo] + B*ind[hi] + D ----
            corr = epool.tile([P, NT * W], f32)
            c1 = scrpool.tile([P, NT * W], f32)
            c2 = scrpool.tile([P, NT * W], f32)
            for t in range(NT):
                tw = slice(t * W, (t + 1) * W)
                lo_v = bnd_ind[:, t * NB:t * NB + W]
                hi_v = bnd_ind[:, t * NB + 1:t * NB + 1 + W]
                nc.vector.tensor_tensor(out=c1[:, tw], in0=bmask[:, t * W:(t + 1) * W],
                                        in1=lo_v, op=mybir.AluOpType.mult)
                nc.vector.tensor_tensor(out=c2[:, tw],
                                        in0=bmask[:, NT * W + t * W:NT * W + (t + 1) * W],
                                        in1=hi_v, op=mybir.AluOpType.mult)
                nc.vector.tensor_tensor(out=c1[:, tw], in0=c1[:, tw], in1=c2[:, tw],
                                        op=mybir.AluOpType.add)
                nc.vector.tensor_copy(c2[:, tw], bmask[:, 2 * NT * W + t * W:2 * NT * W + (t + 1) * W])
                nc.vector.tensor_tensor(out=corr[:, tw], in0=c1[:, tw], in1=c2[:, tw],
                                        op=mybir.AluOpType.add)

            # ---- npos = ssum + halfn - corr; m0, m19 ----
            npos = epool.tile([P, NT * W], f32)
            nc.vector.tensor_tensor(out=npos[:], in0=ssum[:], in1=corr[:],
                                    op=mybir.AluOpType.subtract)
            halfn_v = fconst[:].copy()
            _set_ap(halfn_v, [tuple(halfn_v.ap[0]), (0, NT), (1, W)])
            nc.vector.tensor_tensor(out=npos[:].rearrange("p (t w) -> p t w", w=W),
                                    in0=npos[:].rearrange("p (t w) -> p t w", w=W),
                                    in1=halfn_v, op=mybir.AluOpType.add)
            nc.vector.tensor_scalar_min(npos[:], npos[:], float(KNN))
            m0 = epool.tile([P, NT], f32)
            nc.vector.tensor_reduce(out=m0[:], in_=npos[:].rearrange("p (t w) -> p t w", w=W),
                                    axis=mybir.AxisListType.X, op=mybir.AluOpType.add)
            m19 = epool.tile([P, NT], f32)
            nc.vector.tensor_scalar(out=m19[:], in0=m0[:], scalar1=-1.0,
                                    scalar2=float(KNN * W), op0=mybir.AluOpType.mult,
                                    op1=mybir.AluOpType.add)

            # ---- rec = psi_j(p); nbs; cumsums; prec; ap ----
            qoff = W

            def quant_bc(k):
                v = fconst[:].copy()
                v.offset = v.offset + qoff + k * NQ
                _set_ap(v, [tuple(v.ap[0]), (0, NT), (1, NQ)])
                return v
            pbc = pval[:].copy()
            _set_ap(pbc, [tuple(pbc.ap[0]), (1, NT), (0, NQ)])

            q1 = epool.tile([P, NT * NQ], f32)
            q2 = epool.tile([P, NT * NQ], f32)
            rec = epool.tile([P, NT * NQ], f32)
            nbs = epool.tile([P, NT * NQ], f32)
            q1v = q1[:].rearrange("p (t q) -> p t q", q=NQ)
            q2v = q2[:].rearrange("p (t q) -> p t q", q=NQ)
            nc.vector.tensor_tensor(out=q1v, in0=pbc, in1=quant_bc(0), op=mybir.AluOpType.mult)
            nc.vector.tensor_tensor(out=q1v, in0=q1v, in1=quant_bc(1), op=mybir.AluOpType.add)
            nc.vector.tensor_tensor(out=q2v, in0=pbc, in1=quant_bc(2), op=mybir.AluOpType.mult)
            nc.vector.tensor_tensor(out=q2v, in0=q2v, in1=quant_bc(3), op=mybir.AluOpType.add)
            nc.vector.tensor_tensor(out=q1[:], in0=q1[:], in1=q2[:], op=mybir.AluOpType.min)
            nc.vector.tensor_scalar_max(rec[:], q1[:], 0.0)

            nc.vector.tensor_copy(nbs[:], rec[:])
            nbs0 = nbs[:, 0:NT * NQ:NQ]
            nc.vector.tensor_tensor(out=nbs0, in0=nbs0, in1=m0[:], op=mybir.AluOpType.add)
            nbs19 = nbs[:, NQ - 1:NT * NQ:NQ]
            nc.vector.tensor_tensor(out=nbs19, in0=nbs19, in1=m19[:], op=mybir.AluOpType.add)

            cumr = epool.tile([P, NT * NQ], f32)
            cumn = epool.tile([P, NT * NQ], f32)
            for t in range(NT):
                sl = slice(t * NQ, (t + 1) * NQ)
                nc.vector.tensor_tensor_scan(
                    out=cumr[:, sl], data0=rec[:, sl], data1=rec[:, sl],
                    initial=0.0, op0=mybir.AluOpType.add, op1=mybir.AluOpType.bypass)
                nc.vector.tensor_tensor_scan(
                    out=cumn[:, sl], data0=nbs[:, sl], data1=nbs[:, sl],
                    initial=0.0, op0=mybir.AluOpType.add, op1=mybir.AluOpType.bypass)
            nc.vector.tensor_scalar_add(cumn[:], cumn[:], 1e-16)
            cninv = epool.tile([P, NT * NQ], f32)
            nc.vector.reciprocal(cninv[:], cumn[:])
            prec = epool.tile([P, NT * NQ], f32)
            nc.vector.tensor_tensor(out=prec[:], in0=cumr[:], in1=cninv[:],
                                    op=mybir.AluOpType.mult)

            srec = epool.tile([P, NT], f32)
            nc.vector.tensor_reduce(out=srec[:], in_=rec[:].rearrange("p (t q) -> p t q", q=NQ),
                                    axis=mybir.AxisListType.X, op=mybir.AluOpType.add)
            sinv = epool.tile([P, NT], f32)
            nc.vector.reciprocal(sinv[:], srec[:])

            apraw = epool.tile([P, NT], f32)
            apterm = epool.tile([P, NT * NQ], f32)
            nc.vector.tensor_tensor(out=apterm[:], in0=prec[:], in1=rec[:],
                                    op=mybir.AluOpType.mult)
            nc.vector.tensor_reduce(out=apraw[:],
                                    in_=apterm[:].rearrange("p (t q) -> p t q", q=NQ),
                                    axis=mybir.AxisListType.X, op=mybir.AluOpType.add)
            apout = epool.tile([P, NT], f32)
            nc.vector.tensor_tensor(out=apout[:], in0=apraw[:], in1=sinv[:],
                                    op=mybir.AluOpType.mult)
            nc.sync.dma_start(out_d.ap()[:], apout[:])

    nc.compile()
    return nc


def _uniform_windows(windows):
    if not windows:
        return False
    ns = {n for _, n in windows}
    if len(ns) != 1:
        return False
    n0 = windows[0][1]
    if n0 + 1 > 512:
        return False
    if len(windows) > 1:
        steps = {windows[i + 1][0] - windows[i][0] for i in range(len(windows) - 1)}
        if steps != {n0}:
            return False
    return True


def _act_tiles(b):
    # static ACT/DVE split of the NCT col-tiles (tune ratio from traces)
    NCT = b // P
    return {ct for ct in range(NCT) if ct % 8 < 5}


def _host_inputs_v2(anc, pos, windows, b, d, act_tiles):
    W = len(windows)
    S = b // N_CORES
    NT = S // P
    NCT = b // P
    NB = W + 1
    w1, b1, w2, b2 = _quant_coeffs()

    pos_bf = pos.astype(ml_dtypes.bfloat16)
    posT = np.ascontiguousarray(pos_bf.T)

    # selector weights [P, NCT*W]: col k of tile ct belongs to window w
    # (cols lo_w..lo_w+n inclusive); 0.5 for sign-tiles, 1.0 for ind-tiles
    selw = np.zeros((P, NCT * W), np.float32)
    halfn = np.zeros(W, np.float32)
    for ct in range(NCT):
        scale = 0.5 if ct in act_tiles else 1.0
        cols = np.arange(ct * P, (ct + 1) * P)
        for w, (lo, n) in enumerate(windows):
            inwin = (cols >= lo) & (cols <= lo + n)
            selw[:, ct * W + w] = inwin * scale
            if ct in act_tiles:
                halfn[w] += inwin.sum() * 0.5
    quant = np.concatenate([w1, b1, w2, b2]).astype(np.float32)
    fconst = np.concatenate([
        np.tile(halfn[None, :], (P, 1)),
        np.tile(quant[None, :], (P, 1))], axis=1).astype(np.float32)
    fid = np.eye(W, dtype=np.float32)

    in_maps = []
    for c in range(N_CORES):
        rows = np.arange(c * S, (c + 1) * S)
        A = np.zeros((S, W), np.float32)
        B = np.zeros((S, W), np.float32)
        D = np.zeros((S, W), np.float32)
        for w, (lo, n) in enumerate(windows):
            hi = lo + n
            A[:, w] = rows < lo
            B[:, w] = rows > hi
            D[:, w] = (rows >= lo) & (rows <= hi)

        def to_ptw(x):
            return np.ascontiguousarray(
                x.reshape(NT, P, W).transpose(1, 0, 2).reshape(P, NT * W))

        bmask = np.concatenate([to_ptw(A), to_ptw(B), to_ptw(D)], axis=1)
        in_maps.append({
            "posT": posT,
            "lhsT": np.ascontiguousarray(pos_bf[c * S:(c + 1) * S].T),
            "selw": selw.astype(ml_dtypes.bfloat16),
            "anc_sh": np.ascontiguousarray(anc[c * S:(c + 1) * S]),
            "pos_sh": np.ascontiguousarray(pos[c * S:(c + 1) * S]),
            "bmask": bmask.astype(ml_dtypes.bfloat16),
            "fconst": fconst,
            "fid16": fid,
        })
    return in_maps


def _host_inputs(anc, pos, windows, b, d):
    """Per-core input maps (the sharding step)."""
    W = len(windows)
    S = b // N_CORES
    NT = S // P
    w1, b1, w2, b2 = _quant_coeffs()

    pos_bf = pos.astype(ml_dtypes.bfloat16)
    posT = np.ascontiguousarray(pos_bf.T)                     # [d, b] bf16

    quant = np.concatenate([w1, b1, w2, b2]).astype(np.float32)  # [4*NQ]

    in_maps = []
    for c in range(N_CORES):
        rows = np.arange(c * S, (c + 1) * S)
        # masks per (row, window)
        A = np.zeros((S, W), np.float32)
        B = np.zeros((S, W), np.float32)
        D = np.zeros((S, W), np.float32)
        halfn = np.zeros((S, W), np.float32)
        for w, (lo, n) in enumerate(windows):
            hi = lo + n
            A[:, w] = rows < lo
            B[:, w] = rows > hi
            D[:, w] = (rows >= lo) & (rows <= hi)
            halfn[:, w] = n / 2.0

        def to_ptw(x):  # [S, W] -> [P, NT*W]
            return np.ascontiguousarray(
                x.reshape(NT, P, W).transpose(1, 0, 2).reshape(P, NT * W))

        bmask = np.concatenate([to_ptw(A), to_ptw(B), to_ptw(D)], axis=1)
        fconst = np.concatenate(
            [to_ptw(halfn), np.tile(quant[None, :], (P, 1))], axis=1).astype(np.float32)

        in_maps.append({
            "posT": posT,
            "lhsT": np.ascontiguousarray(pos_bf[c * S:(c + 1) * S].T),
            "anc_sh": np.ascontiguousarray(anc[c * S:(c + 1) * S]),
            "pos_sh": np.ascontiguousarray(pos[c * S:(c + 1) * S]),
            "bmask": bmask.astype(ml_dtypes.bfloat16),
            "fconst": fconst,
        })
    return in_maps


def kernel(anc_feat, pos_feat, kpts_crop_ids):
    global LAST_EXEC_NS, LAST_TRACE_PATH, LAST_RESULTS
    from concourse.bass_utils import run_bass_kernel_spmd

    anc = np.asarray(anc_feat, dtype=np.float32)
    pos = np.asarray(pos_feat, dtype=np.float32)
    b, d = pos.shape
    windows = _crop_windows(kpts_crop_ids)
    W = len(windows)
    S = b // N_CORES
    NT = S // P

    use_v2 = _uniform_windows(windows) and b % P == 0 and S % P == 0
    key = (b, d, tuple(windows), use_v2)
    if key not in _GRAPH_CACHE:
        if use_v2:
            _GRAPH_CACHE[key] = _build_graph_v2(b, d, windows, _act_tiles(b))
        else:
            _GRAPH_CACHE[key] = _build_graph(b, d, windows)
    nc = _GRAPH_CACHE[key]

    if use_v2:
        in_maps = _host_inputs_v2(anc, pos, windows, b, d, _act_tiles(b))
    else:
        in_maps = _host_inputs(anc, pos, windows, b, d)
    res = run_bass_kernel_spmd(nc, in_maps, list(range(N_CORES)), trace=TRACE)
    LAST_RESULTS = res
    LAST_EXEC_NS = res.exec_time_ns
    if res.instructions_and_trace is not None:
        LAST_TRACE_PATH = res.instructions_and_trace[1]

    ap = np.empty(b, np.float32)
    for c in range(N_CORES):
        o = np.asarray(res.results[c]["out"], dtype=np.float32)  # [P, NT]
        ap[c * S:(c + 1) * S] = o.T.reshape(S)

    one = np.float32(1.0)
    loss = (one - ap).mean(dtype=np.float32)
    apm = ap.mean(dtype=np.float32)
    return (np.asarray(loss, dtype=np.float32), np.asarray(apm, dtype=np.float32))


# revision 11
# speedup vs baseline: 1.0802x; 1.0117x over previous
"""Trainium2 Bass kernel for nn_APCriterionWeighted (weighted-AP criterion).

Math summary (exact simplifications of the reference, not approximations):
  - sim_w = sim / stop_grad(sim * sim_self) == (1/sim_self) elementwise in
    real arithmetic (verified < 1.2e-7 rel diff in f32 on the fixed inputs).
  - x = 1/b for |b| <= 1 satisfies |x| >= 1, so in the 20-bin quantizer on
    [0, 1] every selected negative lands entirely in bin 0 (if b > 0) or
    bin 19 (if b < 0).  The per-row top-KNN of 1/b over a crop segment picks
    all positive-b entries first, so the negatives' soft-histogram is exactly
    [min(KNN, npos_seg) into bin 0, rest into bin 19] per segment.
  - Therefore per-row AP = f(diag terms, per-segment positive counts), where
    the counts come from the signs of sim_self = pos @ pos.T.

Device work per core (rows sharded 8 ways, data-parallel):
  - sim_self shard = pos_shard @ pos.T on PE (bf16 in, f32 PSUM out)
  - signs on ACT, sliding-window sign-sums + count epilogue + full 20-bin
    AP formula (cumsums, precision/recall, AP) on DVE
  - per-row AP DMA'd out; host computes the two scalar means (the unshard).
"""

import numpy as np
import ml_dtypes

KNN = 20


def _set_ap(ap, pairs):
    import bass_rust
    ap.ap = bass_rust.VecI64Pair(pairs)
    return ap
NQ = 20
N_CORES = 8
P = 128

# module knobs (test.py pokes these; the grading harness just calls kernel())
TRACE = False
LAST_EXEC_NS = None
LAST_TRACE_PATH = None
LAST_RESULTS = None

_GRAPH_CACHE = {}


def _crop_windows(kpts_crop_ids):
    """Replicate the reference's static segment walk.

    Returns list of (lo, n): off-diagonal columns [lo, lo+n) per active crop;
    in actual-column space the window is [lo, lo+n] (n+1 cols) with one
    excluded column clip(i, lo, lo+n) for row i.
    """
    kpts = np.asarray(kpts_crop_ids).astype(np.int64) - 1
    windows = []
    k = 0
    for n in kpts:
        n = int(n)
        if n < 0:
            continue
        if n < KNN:
            k += n
            continue
        windows.append((k, n))
        k += n
    return windows


def _quant_coeffs():
    a = np.float32(NQ - 1)
    w1 = np.full(NQ, -a, np.float32)
    b1 = np.arange(NQ, 0, -1).astype(np.float32)
    w2 = np.full(NQ, a, np.float32)
    b2 = np.arange(2 - NQ, 2, 1).astype(np.float32)
    w1[0] = 0.0
    b1[0] = 1.0
    w2[-1] = 0.0
    b2[-1] = 1.0
    return w1, b1, w2, b2


def _build_graph(b, d, windows):
    """Build the SPMD Bass/Tile graph (identical across cores)."""
    import concourse.bass as bass
    import concourse.tile as tile
    from concourse import bacc, mybir

    W = len(windows)
    S = b // N_CORES          # rows per core
    NT = S // P               # 128-row tiles per core
    NCH = (b + 511) // 512    # 512-col chunks of the full row
    assert S % P == 0 and b % 512 == 0

    uniform = len({n for _, n in windows}) == 1
    if uniform:
        n0 = windows[0][1]
        los = [lo for lo, _ in windows]
        steps = {los[i + 1] - los[i] for i in range(W - 1)} if W > 1 else {0}
        uniform = len(steps) <= 1
        lo_step = steps.pop() if W > 1 else 0

    f32 = mybir.dt.float32
    bf16 = mybir.dt.bfloat16

    nc = bacc.Bacc("TRN2", target_bir_lowering=False, debug=False,
                   enable_asserts=True, num_devices=N_CORES)

    posT_d = nc.declare_dram_parameter("posT", [P, b], bf16, isOutput=False)
    lhsT_d = nc.declare_dram_parameter("lhsT", [P, S], bf16, isOutput=False)
    anc_d = nc.declare_dram_parameter("anc_sh", [P, NT * d], f32, isOutput=False)
    pos_d = nc.declare_dram_parameter("pos_sh", [P, NT * d], f32, isOutput=False)
    bmask_d = nc.declare_dram_parameter("bmask", [P, 3 * NT * W], bf16, isOutput=False)
    fconst_d = nc.declare_dram_parameter("fconst", [P, NT * W + 4 * NQ], f32, isOutput=False)
    out_d = nc.declare_dram_parameter("out", [P, NT], f32, isOutput=True)

    with tile.TileContext(nc) as tc:
        with (
            tc.tile_pool(name="const", bufs=1) as cpool,
            tc.tile_pool(name="stage", bufs=4) as spool,
            tc.tile_pool(name="sgn", bufs=2) as gpool,
            tc.tile_pool(name="scr", bufs=2) as scrpool,
            tc.tile_pool(name="ep", bufs=1) as epool,
            tc.tile_pool(name="psum", bufs=8, space=bass.MemorySpace.PSUM) as ppool,
        ):
            # ---- constant loads ----
            posT = cpool.tile([P, b], bf16)
            for j in range(NCH):
                nc.sync.dma_start(posT[:, j * 512:(j + 1) * 512],
                                  posT_d.ap()[:, j * 512:(j + 1) * 512])
            lhsT = cpool.tile([P, S], bf16)
            nc.sync.dma_start(lhsT[:], lhsT_d.ap()[:])
            bmask = cpool.tile([P, 3 * NT * W], bf16)
            nc.sync.dma_start(bmask[:], bmask_d.ap()[:])
            fconst = cpool.tile([P, NT * W + 4 * NQ], f32)
            nc.sync.dma_start(fconst[:], fconst_d.ap()[:])

            # ---- diagonal path (f32, faithful a/(a*b)), batched over tiles ----
            pdiag = epool.tile([P, NT], f32)
            bdiag = epool.tile([P, NT], f32)
            a_all = spool.tile([P, NT * d], f32, tag="diag_a")
            p_all = spool.tile([P, NT * d], f32, tag="diag_p")
            nc.sync.dma_start(a_all[:], anc_d.ap()[:])
            nc.sync.dma_start(p_all[:], pos_d.ap()[:])
            scr = scrpool.tile([P, NT * d], f32, tag="diag_scr")
            nc.vector.tensor_tensor(out=scr[:], in0=a_all[:], in1=p_all[:],
                                    op=mybir.AluOpType.mult)
            nc.vector.tensor_reduce(out=pdiag[:], in_=scr[:].rearrange("p (t k) -> p t k", k=d),
                                    axis=mybir.AxisListType.X, op=mybir.AluOpType.add)
            scr2 = scrpool.tile([P, NT * d], f32, tag="diag_scr")
            nc.vector.tensor_tensor(out=scr2[:], in0=p_all[:], in1=p_all[:],
                                    op=mybir.AluOpType.mult)
            nc.vector.tensor_reduce(out=bdiag[:], in_=scr2[:].rearrange("p (t k) -> p t k", k=d),
                                    axis=mybir.AxisListType.X, op=mybir.AluOpType.add)
            pbprod = epool.tile([P, NT], f32)
            nc.vector.tensor_tensor(out=pbprod[:], in0=pdiag[:], in1=bdiag[:],
                                    op=mybir.AluOpType.mult)
            pbinv = epool.tile([P, NT], f32)
            nc.vector.reciprocal(pbinv[:], pbprod[:])
            pval = epool.tile([P, NT], f32)
            nc.vector.tensor_tensor(out=pval[:], in0=pdiag[:], in1=pbinv[:],
                                    op=mybir.AluOpType.mult)

            # ---- main loop: matmul -> sign -> window sign-sums -> corr ----
            ssum = epool.tile([P, NT * W], f32)
            corr = epool.tile([P, NT * W], f32)
            c1 = scrpool.tile([P, NT * W], f32)
            c2 = scrpool.tile([P, NT * W], f32)
            for t in range(NT):
                sgn_t = gpool.tile([P, b], bf16, tag="sgn")
                for j in range(NCH):
                    ps = ppool.tile([P, 512], f32)
                    nc.tensor.matmul(ps[:], lhsT[:, t * P:(t + 1) * P],
                                     posT[:, j * 512:(j + 1) * 512],
                                     start=True, stop=True)
                    nc.scalar.sign(sgn_t[:, j * 512:(j + 1) * 512], ps[:])

                # sliding-window sign sums
                if uniform:
                    win = sgn_t[:].copy()
                    win.offset = win.offset + los[0]
                    _set_ap(win, [tuple(win.ap[0]), (max(lo_step, 1), W), (1, n0 + 1)])
                    nc.vector.tensor_reduce(
                        out=ssum[:, t * W:(t + 1) * W], in_=win,
                        axis=mybir.AxisListType.X, op=mybir.AluOpType.add)
                else:
                    for w, (lo, n) in enumerate(windows):
                        nc.vector.tensor_reduce(
                            out=ssum[:, t * W + w:t * W + w + 1],
                            in_=sgn_t[:, lo:lo + n + 1],
                            axis=mybir.AxisListType.X, op=mybir.AluOpType.add)

                # corr = A*sgn[lo] + B*sgn[hi] + D   per window
                if uniform:
                    lo_v = sgn_t[:].copy()
                    lo_v.offset = lo_v.offset + los[0]
                    _set_ap(lo_v, [tuple(lo_v.ap[0]), (max(lo_step, 1), W)])
                    hi_v = sgn_t[:].copy()
                    hi_v.offset = hi_v.offset + los[0] + n0
                    _set_ap(hi_v, [tuple(hi_v.ap[0]), (max(lo_step, 1), W)])
                else:
                    # gather columns one by one into a packed scratch
                    lo_pack = scrpool.tile([P, W], bf16, tag="lopack")
                    hi_pack = scrpool.tile([P, W], bf16, tag="hipack")
                    for w, (lo, n) in enumerate(windows):
                        nc.vector.tensor_copy(lo_pack[:, w:w + 1], sgn_t[:, lo:lo + 1])
                        nc.vector.tensor_copy(hi_pack[:, w:w + 1], sgn_t[:, lo + n:lo + n + 1])
                    lo_v = lo_pack[:]
                    hi_v = hi_pack[:]
                tw = slice(t * W, (t + 1) * W)
                nc.vector.tensor_tensor(out=c1[:, tw], in0=bmask[:, t * W:(t + 1) * W],
                                        in1=lo_v, op=mybir.AluOpType.mult)
                nc.vector.tensor_tensor(out=c2[:, tw],
                                        in0=bmask[:, NT * W + t * W:NT * W + (t + 1) * W],
                                        in1=hi_v, op=mybir.AluOpType.mult)
                nc.vector.tensor_tensor(out=c1[:, tw], in0=c1[:, tw], in1=c2[:, tw],
                                        op=mybir.AluOpType.add)
                # + D (bf16 mask -> f32 add via copy-widen through c2)
                nc.vector.tensor_copy(c2[:, tw], bmask[:, 2 * NT * W + t * W:2 * NT * W + (t + 1) * W])
                nc.vector.tensor_tensor(out=corr[:, tw], in0=c1[:, tw], in1=c2[:, tw],
                                        op=mybir.AluOpType.add)

            # ---- counts ----
            raw = epool.tile([P, NT * W], f32)
            nc.vector.tensor_tensor(out=raw[:], in0=ssum[:], in1=corr[:],
                                    op=mybir.AluOpType.subtract)
            npos = epool.tile([P, NT * W], f32)
            halfn_v = fconst[:, 0:NT * W]
            nc.vector.scalar_tensor_tensor(out=npos[:], in0=raw[:], scalar=0.5,
                                           in1=halfn_v, op0=mybir.AluOpType.mult,
                                           op1=mybir.AluOpType.add)
            nc.vector.tensor_scalar_min(npos[:], npos[:], float(KNN))
            m0 = epool.tile([P, NT], f32)
            nc.vector.tensor_reduce(out=m0[:], in_=npos[:].rearrange("p (t w) -> p t w", w=W),
                                    axis=mybir.AxisListType.X, op=mybir.AluOpType.add)
            m19 = epool.tile([P, NT], f32)
            nc.vector.tensor_scalar(out=m19[:], in0=m0[:], scalar1=-1.0,
                                    scalar2=float(KNN * W), op0=mybir.AluOpType.mult,
                                    op1=mybir.AluOpType.add)

            # ---- rec = psi_j(p)  [P, NT*NQ] ----
            qoff = NT * W
            def quant_bc(k):
                v = fconst[:].copy()
                v.offset = v.offset + qoff + k * NQ
                _set_ap(v, [tuple(v.ap[0]), (0, NT), (1, NQ)])
                return v
            pbc = pval[:].copy()
            _set_ap(pbc, [tuple(pbc.ap[0]), (1, NT), (0, NQ)])

            q1 = epool.tile([P, NT * NQ], f32)
            q2 = epool.tile([P, NT * NQ], f32)
            rec = epool.tile([P, NT * NQ], f32)
            nbs = epool.tile([P, NT * NQ], f32)
            q1v = q1[:].rearrange("p (t q) -> p t q", q=NQ)
            q2v = q2[:].rearrange("p (t q) -> p t q", q=NQ)
            nc.vector.tensor_tensor(out=q1v, in0=pbc, in1=quant_bc(0), op=mybir.AluOpType.mult)
            nc.vector.tensor_tensor(out=q1v, in0=q1v, in1=quant_bc(1), op=mybir.AluOpType.add)
            nc.vector.tensor_tensor(out=q2v, in0=pbc, in1=quant_bc(2), op=mybir.AluOpType.mult)
            nc.vector.tensor_tensor(out=q2v, in0=q2v, in1=quant_bc(3), op=mybir.AluOpType.add)
            nc.vector.tensor_tensor(out=q1[:], in0=q1[:], in1=q2[:], op=mybir.AluOpType.min)
            nc.vector.tensor_scalar_max(rec[:], q1[:], 0.0)

            nc.vector.tensor_copy(nbs[:], rec[:])
            nbs0 = nbs[:, 0:NT * NQ:NQ]
            nc.vector.tensor_tensor(out=nbs0, in0=nbs0, in1=m0[:], op=mybir.AluOpType.add)
            nbs19 = nbs[:, NQ - 1:NT * NQ:NQ]
            nc.vector.tensor_tensor(out=nbs19, in0=nbs19, in1=m19[:], op=mybir.AluOpType.add)

            # ---- cumsums, prec, ap ----
            cumr = epool.tile([P, NT * NQ], f32)
            cumn = epool.tile([P, NT * NQ], f32)
            for t in range(NT):
                sl = slice(t * NQ, (t + 1) * NQ)
                nc.vector.tensor_tensor_scan(
                    out=cumr[:, sl], data0=rec[:, sl], data1=rec[:, sl],
                    initial=0.0, op0=mybir.AluOpType.add, op1=mybir.AluOpType.bypass)
                nc.vector.tensor_tensor_scan(
                    out=cumn[:, sl], data0=nbs[:, sl], data1=nbs[:, sl],
                    initial=0.0, op0=mybir.AluOpType.add, op1=mybir.AluOpType.bypass)
            nc.vector.tensor_scalar_add(cumn[:], cumn[:], 1e-16)
            cninv = epool.tile([P, NT * NQ], f32)
            nc.vector.reciprocal(cninv[:], cumn[:])
            prec = epool.tile([P, NT * NQ], f32)
            nc.vector.tensor_tensor(out=prec[:], in0=cumr[:], in1=cninv[:],
                                    op=mybir.AluOpType.mult)

            srec = epool.tile([P, NT], f32)
            nc.vector.tensor_reduce(out=srec[:], in_=rec[:].rearrange("p (t q) -> p t q", q=NQ),
                                    axis=mybir.AxisListType.X, op=mybir.AluOpType.add)
            sinv = epool.tile([P, NT], f32)
            nc.vector.reciprocal(sinv[:], srec[:])

            apraw = epool.tile([P, NT], f32)
            apterm = epool.tile([P, NT * NQ], f32)
            nc.vector.tensor_tensor(out=apterm[:], in0=prec[:], in1=rec[:],
                                    op=mybir.AluOpType.mult)
            nc.vector.tensor_reduce(out=apraw[:],
                                    in_=apterm[:].rearrange("p (t q) -> p t q", q=NQ),
                                    axis=mybir.AxisListType.X, op=mybir.AluOpType.add)
            apout = epool.tile([P, NT], f32)
            nc.vector.tensor_tensor(out=apout[:], in0=apraw[:], in1=sinv[:],
                                    op=mybir.AluOpType.mult)
            nc.sync.dma_start(out_d.ap()[:], apout[:])

    nc.compile()
    return nc



def _build_graph_v2(b, d, windows, act_tiles):
    """Transposed-counts design (uniform windows, width n+1 = 256, lo step 255).

    Per core: Gt col-tiles [128 cols, S rows] on PE; sign(ACT)/is_gt(DVE) per
    col-tile; per-window positive counts via a selector matmul on PE
    (contraction over the col partitions), accumulated in one PSUM bank;
    boundary-column corrections from a tiny strided matmul; epilogue row-major.
    """
    import concourse.bass as bass
    import concourse.tile as tile
    from concourse import bacc, mybir

    W = len(windows)
    S = b // N_CORES
    NT = S // P
    NCT = b // P                  # col-tiles
    n0 = windows[0][1]
    lo0 = windows[0][0]
    lo_step = windows[1][0] - windows[0][0] if W > 1 else 1
    NB = W + 1                    # boundary cols (shared lo/hi)

    f32 = mybir.dt.float32
    bf16 = mybir.dt.bfloat16

    nc = bacc.Bacc("TRN2", target_bir_lowering=False, debug=False,
                   enable_asserts=True, num_devices=N_CORES)

    posT_d = nc.declare_dram_parameter("posT", [P, b], bf16, isOutput=False)
    lhsT_d = nc.declare_dram_parameter("lhsT", [P, S], bf16, isOutput=False)
    selw_d = nc.declare_dram_parameter("selw", [P, NCT * W], bf16, isOutput=False)
    anc_d = nc.declare_dram_parameter("anc_sh", [P, NT * d], f32, isOutput=False)
    pos_d = nc.declare_dram_parameter("pos_sh", [P, NT * d], f32, isOutput=False)
    bmask_d = nc.declare_dram_parameter("bmask", [P, 2 * NT * W], bf16, isOutput=False)
    fconst_d = nc.declare_dram_parameter("fconst", [P, NT * W + 4 * NQ], f32, isOutput=False)
    fid_d = nc.declare_dram_parameter("fid16", [W, W], f32, isOutput=False)
    out_d = nc.declare_dram_parameter("out", [P, NT], f32, isOutput=True)

    with tile.TileContext(nc) as tc:
        with (
            tc.tile_pool(name="const", bufs=1) as cpool,
            tc.tile_pool(name="stage", bufs=4) as spool,
            tc.tile_pool(name="sgn", bufs=6) as gpool,
            tc.tile_pool(name="scr", bufs=2) as scrpool,
            tc.tile_pool(name="ep", bufs=1) as epool,
            tc.tile_pool(name="psum", bufs=5, space=bass.MemorySpace.PSUM) as ppool,
            tc.tile_pool(name="psacc", bufs=1, space=bass.MemorySpace.PSUM) as papool,
            tc.tile_pool(name="pssm", bufs=1, space=bass.MemorySpace.PSUM) as pspool,
        ):
            # ---- input loads (posT/lhsT/selw first: they gate the PE) ----
            posT = cpool.tile([P, b], bf16)
            for j in range(8):
                cw = b // 8
                nc.sync.dma_start(posT[:, j * cw:(j + 1) * cw],
                                  posT_d.ap()[:, j * cw:(j + 1) * cw])
            lhsT = cpool.tile([P, S], bf16)
            for j in range(2):
                nc.sync.dma_start(lhsT[:, j * (S // 2):(j + 1) * (S // 2)],
                                  lhsT_d.ap()[:, j * (S // 2):(j + 1) * (S // 2)])
            selw = cpool.tile([P, NCT * W], bf16)
            for j in range(4):
                cw = NCT * W // 4
                nc.scalar.dma_start(selw[:, j * cw:(j + 1) * cw],
                                    selw_d.ap()[:, j * cw:(j + 1) * cw])
            bmask = cpool.tile([P, 2 * NT * W], bf16)
            nc.scalar.dma_start(bmask[:], bmask_d.ap()[:])
            fconst = cpool.tile([P, NT * W + 4 * NQ], f32)
            nc.scalar.dma_start(fconst[:], fconst_d.ap()[:])
            fid = cpool.tile([W, W], f32)
            nc.scalar.dma_start(fid[:], fid_d.ap()[:])

            # ---- main col-tile loop: Gt -> sign/ind -> selector matmul ----
            ssumT_ps = papool.tile([W, S], f32)
            for ct in range(NCT):
                ps = ppool.tile([P, S], f32)
                nc.tensor.matmul(ps[:], posT[:, ct * P:(ct + 1) * P], lhsT[:],
                                 start=True, stop=True)
                v_ct = gpool.tile([P, S], bf16, tag="sgnT")
                if ct in act_tiles:
                    nc.scalar.sign(v_ct[:], ps[:])
                else:
                    nc.vector.tensor_scalar(out=v_ct[:], in0=ps[:], scalar1=0.0,
                                            scalar2=None, op0=mybir.AluOpType.is_gt)
                nc.tensor.matmul(ssumT_ps[:], selw[:, ct * W:(ct + 1) * W], v_ct[:],
                                 start=(ct == 0), stop=(ct == NCT - 1))

            # ---- boundary columns (row-major, tiny strided matmul) ----
            bndv = posT[:].copy()
            bndv.offset = bndv.offset + lo0
            _set_ap(bndv, [tuple(bndv.ap[0]), (lo_step, NB)])
            bnd_ind = epool.tile([P, NT * NB], bf16)
            for t in range(NT):
                bps = pspool.tile([P, NB], f32, tag="bnd")
                nc.tensor.matmul(bps[:], lhsT[:, t * P:(t + 1) * P], bndv,
                                 start=True, stop=True)
                nc.vector.tensor_scalar(out=bnd_ind[:, t * NB:(t + 1) * NB],
                                        in0=bps[:], scalar1=0.0, scalar2=None,
                                        op0=mybir.AluOpType.is_gt)

            # ---- counts back to row-major: PSUM -> SBUF -> PE transpose ----
            ssumT_sb = epool.tile([W, S], f32)
            nc.vector.tensor_copy(ssumT_sb[:], ssumT_ps[:])
            ssum = epool.tile([P, NT * W], f32)
            for t in range(NT):
                tps = pspool.tile([P, W], f32, tag="ctr")
                nc.tensor.transpose(tps[:], ssumT_sb[:, t * P:(t + 1) * P], fid[:])
                nc.vector.tensor_copy(ssum[:, t * W:(t + 1) * W], tps[:])

            # ---- diagonal path (f32, faithful a/(a*b)), batched over tiles ----
            pdiag = epool.tile([P, NT], f32)
            bdiag = epool.tile([P, NT], f32)
            a_all = spool.tile([P, NT * d], f32, tag="diag_a")
            p_all = spool.tile([P, NT * d], f32, tag="diag_p")
            nc.sync.dma_start(a_all[:], anc_d.ap()[:])
            nc.sync.dma_start(p_all[:], pos_d.ap()[:])
            scr = scrpool.tile([P, NT * d], f32, tag="diag_scr")
            nc.vector.tensor_tensor(out=scr[:], in0=a_all[:], in1=p_all[:],
                                    op=mybir.AluOpType.mult)
            nc.vector.tensor_reduce(out=pdiag[:], in_=scr[:].rearrange("p (t k) -> p t k", k=d),
                                    axis=mybir.AxisListType.X, op=mybir.AluOpType.add)
            scr2 = scrpool.tile([P, NT * d], f32, tag="diag_scr")
            nc.vector.tensor_tensor(out=scr2[:], in0=p_all[:], in1=p_all[:],
                                    op=mybir.AluOpType.mult)
            nc.vector.tensor_reduce(out=bdiag[:], in_=scr2[:].rearrange("p (t k) -> p t k", k=d),
                                    axis=mybir.AxisListType.X, op=mybir.AluOpType.add)
            pbprod = epool.tile([P, NT], f32)
            nc.vector.tensor_tensor(out=pbprod[:], in0=pdiag[:], in1=bdiag[:],
                                    op=mybir.AluOpType.mult)
            pbinv = epool.tile([P, NT], f32)
            nc.vector.reciprocal(pbinv[:], pbprod[:])
            pval = epool.tile([P, NT], f32)
            nc.vector.tensor_tensor(out=pval[:], in0=pdiag[:], in1=pbinv[:],
                                    op=mybir.AluOpType.mult)

            # ---- corr' = A*ind[lo] + B*ind[hi] + (D - halfn), batched ----
            corr = epool.tile([P, NT * W], f32)
            c1 = scrpool.tile([P, NT * W], f32)
            lo_v = bnd_ind[:].copy()
            _set_ap(lo_v, [tuple(lo_v.ap[0]), (NB, NT), (1, W)])
            hi_v = bnd_ind[:].copy()
            hi_v.offset = hi_v.offset + 1
            _set_ap(hi_v, [tuple(hi_v.ap[0]), (NB, NT), (1, W)])
            bm3 = lambda k: bmask[:, k * NT * W:(k + 1) * NT * W].rearrange(
                "p (t w) -> p t w", w=W)
            nc.vector.tensor_tensor(out=corr[:].rearrange("p (t w) -> p t w", w=W),
                                    in0=bm3(0), in1=lo_v, op=mybir.AluOpType.mult)
            nc.vector.tensor_tensor(out=c1[:].rearrange("p (t w) -> p t w", w=W),
                                    in0=bm3(1), in1=hi_v, op=mybir.AluOpType.mult)
            nc.vector.tensor_tensor(out=corr[:], in0=corr[:], in1=c1[:],
                                    op=mybir.AluOpType.add)
            nc.vector.tensor_tensor(out=corr[:], in0=corr[:], in1=fconst[:, 0:NT * W],
                                    op=mybir.AluOpType.add)

            # ---- npos = ssum - corr'; m0, m19 ----
            npos = epool.tile([P, NT * W], f32)
            nc.vector.tensor_tensor(out=npos[:], in0=ssum[:], in1=corr[:],
                                    op=mybir.AluOpType.subtract)
            nc.vector.tensor_scalar_min(npos[:], npos[:], float(KNN))
            m0 = epool.tile([P, NT], f32)
            nc.vector.tensor_reduce(out=m0[:], in_=npos[:].rearrange("p (t w) -> p t w", w=W),
                                    axis=mybir.AxisListType.X, op=mybir.AluOpType.add)
            m19 = epool.tile([P, NT], f32)
            nc.vector.tensor_scalar(out=m19[:], in0=m0[:], scalar1=-1.0,
                                    scalar2=float(KNN * W), op0=mybir.AluOpType.mult,
                                    op1=mybir.AluOpType.add)

            # ---- rec = psi_j(p); nbs; cumsums; prec; ap ----
            qoff = NT * W

            def quant_bc(k):
                v = fconst[:].copy()
                v.offset = v.offset + qoff + k * NQ
                _set_ap(v, [tuple(v.ap[0]), (0, NT), (1, NQ)])
                return v
            pbc = pval[:].copy()
            _set_ap(pbc, [tuple(pbc.ap[0]), (1, NT), (0, NQ)])

            q1 = epool.tile([P, NT * NQ], f32)
            q2 = epool.tile([P, NT * NQ], f32)
            rec = epool.tile([P, NT * NQ], f32)
            nbs = epool.tile([P, NT * NQ], f32)
            q1v = q1[:].rearrange("p (t q) -> p t q", q=NQ)
            q2v = q2[:].rearrange("p (t q) -> p t q", q=NQ)
            nc.vector.tensor_tensor(out=q1v, in0=pbc, in1=quant_bc(0), op=mybir.AluOpType.mult)
            nc.vector.tensor_tensor(out=q1v, in0=q1v, in1=quant_bc(1), op=mybir.AluOpType.add)
            nc.vector.tensor_tensor(out=q2v, in0=pbc, in1=quant_bc(2), op=mybir.AluOpType.mult)
            nc.vector.tensor_tensor(out=q2v, in0=q2v, in1=quant_bc(3), op=mybir.AluOpType.add)
            nc.vector.tensor_tensor(out=q1[:], in0=q1[:], in1=q2[:], op=mybir.AluOpType.min)
            nc.vector.tensor_scalar_max(rec[:], q1[:], 0.0)

            nc.vector.tensor_copy(nbs[:], rec[:])
            nbs0 = nbs[:, 0:NT * NQ:NQ]
            nc.vector.tensor_tensor(out=nbs0, in0=nbs0, in1=m0[:], op=mybir.AluOpType.add)
            nbs19 = nbs[:, NQ - 1:NT * NQ:NQ]
            nc.vector.tensor_tensor(out=nbs19, in0=nbs19, in1=m19[:], op=mybir.AluOpType.add)

            cumr = epool.tile([P, NT * NQ], f32)
            cumn = epool.tile([P, NT * NQ], f32)
            for t in range(NT):
                sl = slice(t * NQ, (t + 1) * NQ)
                nc.vector.tensor_tensor_scan(
                    out=cumr[:, sl], data0=rec[:, sl], data1=rec[:, sl],
                    initial=0.0, op0=mybir.AluOpType.add, op1=mybir.AluOpType.bypass)
                nc.vector.tensor_tensor_scan(
                    out=cumn[:, sl], data0=nbs[:, sl], data1=nbs[:, sl],
                    initial=0.0, op0=mybir.AluOpType.add, op1=mybir.AluOpType.bypass)
            nc.vector.tensor_scalar_add(cumn[:], cumn[:], 1e-16)
            cninv = epool.tile([P, NT * NQ], f32)
            nc.vector.reciprocal(cninv[:], cumn[:])
            prec = epool.tile([P, NT * NQ], f32)
            nc.vector.tensor_tensor(out=prec[:], in0=cumr[:], in1=cninv[:],
                                    op=mybir.AluOpType.mult)

            srec = epool.tile([P, NT], f32)
            nc.vector.tensor_reduce(out=srec[:], in_=rec[:].rearrange("p (t q) -> p t q", q=NQ),
                                    axis=mybir.AxisListType.X, op=mybir.AluOpType.add)
            sinv = epool.tile([P, NT], f32)
            nc.vector.reciprocal(sinv[:], srec[:])

            apraw = epool.tile([P, NT], f32)
            apterm = epool.tile([P, NT * NQ], f32)
            nc.vector.tensor_tensor(out=apterm[:], in0=prec[:], in1=rec[:],
                                    op=mybir.AluOpType.mult)
            nc.vector.tensor_reduce(out=apraw[:],
                                    in_=apterm[:].rearrange("p (t q) -> p t q", q=NQ),
                                    axis=mybir.AxisListType.X, op=mybir.AluOpType.add)
            apout = epool.tile([P, NT], f32)
            nc.vector.tensor_tensor(out=apout[:], in0=apraw[:], in1=sinv[:],
                                    op=mybir.AluOpType.mult)
            nc.sync.dma_start(out_d.ap()[:], apout[:])

    nc.compile()
    return nc


def _uniform_windows(windows):
    if not windows:
        return False
    ns = {n for _, n in windows}
    if len(ns) != 1:
        return False
    n0 = windows[0][1]
    if n0 + 1 > 512:
        return False
    if len(windows) > 1:
        steps = {windows[i + 1][0] - windows[i][0] for i in range(len(windows) - 1)}
        if steps != {n0}:
            return False
    return True


def _act_tiles(b):
    # static ACT/DVE split of the NCT col-tiles (tune ratio from traces)
    NCT = b // P
    return {ct for ct in range(NCT) if ct % 4 < 3}


def _host_inputs_v2(anc, pos, windows, b, d, act_tiles):
    W = len(windows)
    S = b // N_CORES
    NT = S // P
    NCT = b // P
    NB = W + 1
    w1, b1, w2, b2 = _quant_coeffs()

    pos_bf = pos.astype(ml_dtypes.bfloat16)
    posT = np.ascontiguousarray(pos_bf.T)

    # selector weights [P, NCT*W]: col k of tile ct belongs to window w
    # (cols lo_w..lo_w+n inclusive); 0.5 for sign-tiles, 1.0 for ind-tiles
    selw = np.zeros((P, NCT * W), np.float32)
    halfn = np.zeros(W, np.float32)
    for ct in range(NCT):
        scale = 0.5 if ct in act_tiles else 1.0
        cols = np.arange(ct * P, (ct + 1) * P)
        for w, (lo, n) in enumerate(windows):
            inwin = (cols >= lo) & (cols <= lo + n)
            selw[:, ct * W + w] = inwin * scale
            if ct in act_tiles:
                halfn[w] += inwin.sum() * 0.5
    quant = np.concatenate([w1, b1, w2, b2]).astype(np.float32)
    fid = np.eye(W, dtype=np.float32)

    in_maps = []
    for c in range(N_CORES):
        rows = np.arange(c * S, (c + 1) * S)
        A = np.zeros((S, W), np.float32)
        B = np.zeros((S, W), np.float32)
        D = np.zeros((S, W), np.float32)
        for w, (lo, n) in enumerate(windows):
            hi = lo + n
            A[:, w] = rows < lo
            B[:, w] = rows > hi
            D[:, w] = (rows >= lo) & (rows <= hi)

        def to_ptw(x):
            return np.ascontiguousarray(
                x.reshape(NT, P, W).transpose(1, 0, 2).reshape(P, NT * W))

        def to_ptd(x):  # [S, d] -> [P, NT*d]
            return np.ascontiguousarray(
                x.reshape(NT, P, d).transpose(1, 0, 2).reshape(P, NT * d))

        bmask = np.concatenate([to_ptw(A), to_ptw(B)], axis=1)
        dhc = to_ptw(D) - np.tile(halfn[None, :], (P, NT))
        fconst = np.concatenate([dhc, np.tile(quant[None, :], (P, 1))],
                                axis=1).astype(np.float32)
        in_maps.append({
            "posT": posT,
            "lhsT": np.ascontiguousarray(pos_bf[c * S:(c + 1) * S].T),
            "selw": selw.astype(ml_dtypes.bfloat16),
            "anc_sh": to_ptd(anc[c * S:(c + 1) * S]),
            "pos_sh": to_ptd(pos[c * S:(c + 1) * S]),
            "bmask": bmask.astype(ml_dtypes.bfloat16),
            "fconst": fconst,
            "fid16": fid,
        })
    return in_maps


def _host_inputs(anc, pos, windows, b, d):
    """Per-core input maps (the sharding step)."""
    W = len(windows)
    S = b // N_CORES
    NT = S // P
    w1, b1, w2, b2 = _quant_coeffs()

    pos_bf = pos.astype(ml_dtypes.bfloat16)
    posT = np.ascontiguousarray(pos_bf.T)                     # [d, b] bf16

    quant = np.concatenate([w1, b1, w2, b2]).astype(np.float32)  # [4*NQ]

    in_maps = []
    for c in range(N_CORES):
        rows = np.arange(c * S, (c + 1) * S)
        # masks per (row, window)
        A = np.zeros((S, W), np.float32)
        B = np.zeros((S, W), np.float32)
        D = np.zeros((S, W), np.float32)
        halfn = np.zeros((S, W), np.float32)
        for w, (lo, n) in enumerate(windows):
            hi = lo + n
            A[:, w] = rows < lo
            B[:, w] = rows > hi
            D[:, w] = (rows >= lo) & (rows <= hi)
            halfn[:, w] = n / 2.0

        def to_ptw(x):  # [S, W] -> [P, NT*W]
            return np.ascontiguousarray(
                x.reshape(NT, P, W).transpose(1, 0, 2).reshape(P, NT * W))

        bmask = np.concatenate([to_ptw(A), to_ptw(B), to_ptw(D)], axis=1)
        fconst = np.concatenate(
            [to_ptw(halfn), np.tile(quant[None, :], (P, 1))], axis=1).astype(np.float32)

        def to_ptd(x):  # [S, d] -> [P, NT*d]
            return np.ascontiguousarray(
                x.reshape(NT, P, d).transpose(1, 0, 2).reshape(P, NT * d))

        in_maps.append({
            "posT": posT,
            "lhsT": np.ascontiguousarray(pos_bf[c * S:(c + 1) * S].T),
            "anc_sh": to_ptd(anc[c * S:(c + 1) * S]),
            "pos_sh": to_ptd(pos[c * S:(c + 1) * S]),
            "bmask": bmask.astype(ml_dtypes.bfloat16),
            "fconst": fconst,
        })
    return in_maps


def kernel(anc_feat, pos_feat, kpts_crop_ids):
    global LAST_EXEC_NS, LAST_TRACE_PATH, LAST_RESULTS
    from concourse.bass_utils import run_bass_kernel_spmd

    anc = np.asarray(anc_feat, dtype=np.float32)
    pos = np.asarray(pos_feat, dtype=np.float32)
    b, d = pos.shape
    windows = _crop_windows(kpts_crop_ids)
    W = len(windows)
    S = b // N_CORES
    NT = S // P

    use_v2 = _uniform_windows(windows) and b % P == 0 and S % P == 0
    key = (b, d, tuple(windows), use_v2)
    if key not in _GRAPH_CACHE:
        if use_v2:
            _GRAPH_CACHE[key] = _build_graph_v2(b, d, windows, _act_tiles(b))
        else:
            _GRAPH_CACHE[key] = _build_graph(b, d, windows)
    nc = _GRAPH_CACHE[key]

    if use_v2:
        in_maps = _host_inputs_v2(anc, pos, windows, b, d, _act_tiles(b))
    else:
        in_maps = _host_inputs(anc, pos, windows, b, d)
    res = run_bass_kernel_spmd(nc, in_maps, list(range(N_CORES)), trace=TRACE)
    LAST_RESULTS = res
    LAST_EXEC_NS = res.exec_time_ns
    if res.instructions_and_trace is not None:
        LAST_TRACE_PATH = res.instructions_and_trace[1]

    ap = np.empty(b, np.float32)
    for c in range(N_CORES):
        o = np.asarray(res.results[c]["out"], dtype=np.float32)  # [P, NT]
        ap[c * S:(c + 1) * S] = o.T.reshape(S)

    one = np.float32(1.0)
    loss = (one - ap).mean(dtype=np.float32)
    apm = ap.mean(dtype=np.float32)
    return (np.asarray(loss, dtype=np.float32), np.asarray(apm, dtype=np.float32))
